# revision 47
# baseline (speedup 1.0000x reference)
"""Trainium2 Bass kernel for nn_EncoderLayer (D=512, H=8, DK=64, DF=2048, B=2, S=2048).

Strategy v2 (8 NeuronCores, batch-split tensor parallel):
  - Core c handles batch b=c//4 and heads (2*(c%4), 2*(c%4)+1). Groups of 4
    cores (one per die pair) cover one batch; the post-attention exchange is
    a 4-rank AllToAll within the group (same-die links, half the bytes).
  - All data bf16 (fp32 accumulation in PSUM); weights are pre-swizzled on
    the host into the exact SBUF layout so every DMA moves contiguous >=1KB
    lines in a handful of large transfers.
  - Attention: q/k kept feature-major (zero-padded to K=128); scores in
    PSUM -> exp on ACT -> AV matmul in token-major orientation
    (lhsT=exp-scores) so the softmax denominator lands as a per-partition
    column: reciprocal + tensor_scalar normalize are cheap, then a PE
    transpose produces the feature-major shard for the exchange.
  - Each head's output is shipped in its own AllToAll; the first one
    overlaps the second head's attention. After the exchange: Wo + LN1 +
    FFN + LN2 on this core's 512 tokens.
"""

import numpy as np

import concourse.bass as bass
import concourse.tile as tile
from concourse import mybir
from concourse.bass_utils import run_bass_kernel_spmd
from concourse.masks import make_identity

F32 = mybir.dt.float32
F32R = mybir.dt.float32r
BF16 = mybir.dt.bfloat16
AF = mybir.ActivationFunctionType
ALU = mybir.AluOpType

B, S, D, H, DK, DF = 2, 2048, 512, 8, 64, 2048
N_CORES = 8
NSH = 512  # tokens per core
EPS = 1e-5
GROUPS = [list(range(N_CORES))]
HSH = 256  # tokens per core per batch (shard = 256 of b0 + 256 of b1)

# ---------------------------------------------------------------------------
# Workaround: this walrus build rejects >1 sem wait on CTRL-type (drain)
# instructions. Split the TileContext tail-drain waits onto dedicated
# single-wait nops; the all-engine barrier right after keeps semantics.


def _split_excess_waits(nc, max_waits=1):
    for fn in nc.m.functions:
        for bb in fn.blocks:
            new_list = []
            for inst in bb.instructions:
                si = inst.sync_info
                waits = list(si.on_wait) if si is not None and si.on_wait else []
                if len(waits) > max_waits:
                    keep = waits[:max_waits]
                    extra = waits[max_waits:]
                    si.on_wait = keep
                    for w in extra:
                        nop = mybir.InstNoOp(name=f"I-waitnop-{nc.next_id()}")
                        nop.engine = inst.engine
                        nop.sync_info = mybir.SyncInfo(on_wait=[w], on_update=[])
                        new_list.append(nop)
                new_list.append(inst)
            bb.instructions = new_list


def _bcast_ap(handle, parts, n):
    """AP reading a 1-D DRAM tensor broadcast across `parts` partitions."""
    a = handle[:]
    return bass.AP(tensor=a.tensor, offset=a.offset, ap=[[0, parts], [1, n]])


def _v():
    import os

    return os.environ.get("KV2_VARIANT", "full")


def build_nc():
    nc = bass.Bass(target_bir_lowering=False)

    # ---- kernel I/O (per core; host pre-swizzles to SBUF layouts) ----
    xt_h = nc.dram_tensor("xt", [D, S], BF16, kind="ExternalInput")
    wqk_h = nc.dram_tensor("wqk", [128, 1024], BF16, kind="ExternalInput")
    wv_h = nc.dram_tensor("wv", [128, 512], BF16, kind="ExternalInput")
    # Wo split by even/odd head rows so both Wo passes use K=64 at offset 0
    woe_h = nc.dram_tensor("woe", [64, 2048], BF16, kind="ExternalInput")
    woo_h = nc.dram_tensor("woo", [64, 2048], BF16, kind="ExternalInput")
    w1_h = nc.dram_tensor("w1", [128, 8192], BF16, kind="ExternalInput")
    w2_h = nc.dram_tensor("w2", [128, 8192], BF16, kind="ExternalInput")
    xsb_h = nc.dram_tensor("xsb", [NSH, D], BF16, kind="ExternalInput")
    cst_h = nc.dram_tensor("cst", [128, 18], F32, kind="ExternalInput")
    bv_h = nc.dram_tensor("bv", [128, 128], F32, kind="ExternalInput")
    g1_h = nc.dram_tensor("g1", [D], F32, kind="ExternalInput")
    g2_h = nc.dram_tensor("g2", [D], F32, kind="ExternalInput")
    be2_h = nc.dram_tensor("be2", [D], F32, kind="ExternalInput")
    # bb2 = b2 + ln1_b (both added to the pre-LN2 sum via the ones matmul)
    b2_h = nc.dram_tensor("b2", [D], F32R, kind="ExternalInput")
    out_h = nc.dram_tensor("out_shard", [NSH, D], F32, kind="ExternalOutput")

    with tile.TileContext(nc) as tc:
        with (
            tc.tile_pool(name="consts", bufs=1) as consts,
            tc.tile_pool(name="qk", bufs=1) as qk_pool,
            tc.tile_pool(name="vaug", bufs=16) as v_pool,
            tc.tile_pool(name="et", bufs=5) as et_pool,
            tc.tile_pool(name="otn", bufs=2) as ot_pool,
            tc.tile_pool(name="oc", bufs=1) as oc_pool,
            tc.tile_pool(name="hh", bufs=1) as h_pool,
            tc.tile_pool(name="f1p", bufs=3) as f1_pool,
            tc.tile_pool(name="tmps", bufs=2) as tmp_pool,
            tc.tile_pool(name="small", bufs=4) as small,
            tc.tile_pool(name="psc", bufs=2, space="PSUM") as psc,
            tc.tile_pool(name="pacc", bufs=4, space="PSUM") as pacc,
            tc.tile_pool(name="dram", bufs=1, space="DRAM") as dram,
        ):
            # ---- input streams. sync queue: big loads in priority order;
            # gpsimd queue: small consts.
            wqk_sb = consts.tile([128, 1024], BF16, tag="wqk")
            nc.sync.dma_start(out=wqk_sb, in_=wqk_h[:, :])
            xt = []
            for dd in range(4):
                t_ = consts.tile([128, S], BF16, tag=f"xt{dd}")
                eng = nc.sync if dd % 2 == 0 else nc.scalar
                eng.dma_start(out=t_, in_=xt_h[128 * dd : 128 * (dd + 1), :])
                xt.append(t_)
            wv_sb = consts.tile([128, 512], BF16, tag="wv")
            nc.sync.dma_start(out=wv_sb, in_=wv_h[:, :])
            woe_sb = consts.tile([64, 2048], BF16, tag="woe")
            nc.sync.dma_start(out=woe_sb, in_=woe_h[:, :])
            woo_sb = consts.tile([64, 2048], BF16, tag="woo")
            nc.sync.dma_start(out=woo_sb, in_=woo_h[:, :])
            w1_sb = consts.tile([128, 8192], BF16, tag="w1")
            nc.sync.dma_start(out=w1_sb, in_=w1_h[:, :])
            w2_sb = consts.tile([128, 8192], BF16, tag="w2")
            nc.sync.dma_start(out=w2_sb, in_=w2_h[:, :])
            xsbo = []
            for i in range(4):
                t_ = consts.tile([128, D], BF16, tag=f"xsbo{i}")
                nc.sync.dma_start(out=t_, in_=xsb_h[128 * i : 128 * (i + 1), :])
                xsbo.append(t_)

            cst_sb = consts.tile([128, 18], F32, tag="cst")
            nc.gpsimd.dma_start(out=cst_sb, in_=cst_h[:, :])
            bv_sb = consts.tile([128, 128], F32, tag="bv")
            nc.gpsimd.dma_start(out=bv_sb, in_=bv_h[:, :])
            g1_t = consts.tile([128, D], F32, tag="g1_t")
            g2_t = consts.tile([128, D], F32, tag="g2_t")
            be2_t = consts.tile([128, D], F32, tag="be2_t")
            for t_sb, h_d in ((g1_t, g1_h), (g2_t, g2_h), (be2_t, be2_h)):
                nc.gpsimd.dma_start(out=t_sb, in_=_bcast_ap(h_d, 128, D))
            b2r = consts.tile([1, D], F32R, tag="b2r")
            nc.gpsimd.dma_start(out=b2r, in_=b2_h[:].rearrange("(o d) -> o d", o=1))

            ident = consts.tile([128, 128], BF16)
            make_identity(nc, ident)
            eps_t = consts.tile([128, 1], F32)
            nc.vector.memset(eps_t, EPS)
            ones128 = consts.tile([1, 128], F32R)
            nc.vector.memset(ones128[:].bitcast(F32), 1.0)
            warm_src = consts.tile([128, 512], BF16, tag="warm")
            nc.vector.memset(warm_src, 0.25)

            def pe_warm(n, name, pin_after=None):
                # dummy matmuls to hold the PE HAM clock-gate open across
                # windows where real matmul work is briefly unavailable
                wp = psc.tile([128, 512], F32, tag="sc", name=f"warm_{name}")
                first = None
                for k in range(n):
                    mm = nc.tensor.matmul(
                        wp,
                        lhsT=warm_src[:, 0:128],
                        rhs=warm_src,
                        start=True,
                        stop=True,
                    )
                    if first is None:
                        first = mm
                if pin_after is not None and first is not None:
                    tile.add_dep_helper(
                        first.ins, pin_after.ins, sync=True,
                        reason="keep PE warm only after the preceding block",
                    )

            pe_warm(18, "boot")

            # ---- QKV projections for both heads of this core ----
            # qT_u/kT_u: [128, S] bf16, rows 0-63 = head u's projection,
            # rows 64-127 zeroed so score matmuls run with K=128.
            qT = [
                qk_pool.tile([128, S], BF16, tag=f"qT{u}", name=f"qT{u}")
                for u in range(2)
            ]
            kT = [
                qk_pool.tile([128, S], BF16, tag=f"kT{u}", name=f"kT{u}")
                for u in range(2)
            ]
            for t_ in qT + kT:
                nc.gpsimd.memset(t_[64:128, :], 0.0)

            for s4 in range(4):
                for qk, dst, bcol in ((0, qT, 0), (1, kT, 1)):
                    ps = psc.tile([128, 512], F32, tag="sc")
                    for dd in range(4):
                        nc.tensor.matmul(
                            ps,
                            lhsT=wqk_sb[:, 256 * dd + 128 * qk : 256 * dd + 128 * (qk + 1)],
                            rhs=xt[dd][:, 512 * s4 : 512 * (s4 + 1)],
                            start=(dd == 0),
                            stop=(dd == 3),
                        )
                    # head 0 copy+bias on DVE, head 1 on ACT — parallel engines
                    nc.vector.tensor_scalar_add(
                        dst[0][0:64, 512 * s4 : 512 * (s4 + 1)],
                        ps[0:64, :],
                        cst_sb[0:64, bcol : bcol + 1],
                    )
                    nc.scalar.activation(
                        out=dst[1][0:64, 512 * s4 : 512 * (s4 + 1)],
                        in_=ps[64:128, :],
                        func=AF.Identity,
                        bias=cst_sb[64:128, bcol : bcol + 1],
                        scale=1.0,
                    )

            # v_aug[t]: [128 tokens, 130] = [v_h0 (64) | 1 | v_h1 (64) | 1]
            v_aug = []
            for t in range(16):
                va = v_pool.tile([128, 130], BF16, tag="vaug", name=f"va{t}")
                va_v = va[:].rearrange("p (u c) -> p u c", c=65)
                nc.gpsimd.memset(va_v[:, :, 64:65], 1.0)
                psv = pacc.tile([128, 128], F32, tag="acc", name=f"psv{t}")
                for dd in range(4):
                    nc.tensor.matmul(
                        psv,
                        lhsT=xt[dd][:, 128 * t : 128 * (t + 1)],
                        rhs=wv_sb[:, 128 * dd : 128 * (dd + 1)],
                        start=(dd == 0),
                        stop=(dd == 3),
                    )
                nc.vector.tensor_tensor(
                    out=va_v[:, :, 0:64],
                    in0=psv[:].rearrange("p (u c) -> p u c", c=64),
                    in1=bv_sb[:].rearrange("p (u c) -> p u c", c=64),
                    op=ALU.add,
                )
                v_aug.append(va)

            # per-unit exchange buffers: 8 blocks of [64 feats, 256 tokens]
            send_h = [dram.tile([512, 256], BF16, name=f"send{u}") for u in range(2)]
            recv_h = [dram.tile([512, 256], BF16, name=f"recv{u}") for u in range(2)]

            # ---- attention per head-unit ----
            for u in range(2):
                # o accumulators: 4 PSUM tiles, each holds 4 s-chunks x 65
                # (64 v-cols + denominator from the ones column).
                o_ps = [
                    pacc.tile([128, 260], F32, tag="acc", name=f"ops{u}_{g}")
                    for g in range(4)
                ]
                et_prev = None

                def emit_av(t, et_half):
                    for half in range(2):
                        et_t, is_u16 = et_half[half]
                        for sl in range(8):
                            s_i = 8 * half + sl
                            lhsT = et_t[:, 128 * sl : 128 * (sl + 1)]
                            if is_u16:
                                lhsT = lhsT.bitcast(BF16)
                            nc.tensor.matmul(
                                o_ps[s_i // 4][:, 65 * (s_i % 4) : 65 * (s_i % 4) + 65],
                                lhsT=lhsT,
                                rhs=v_aug[t][:, 65 * u : 65 * (u + 1)],
                                start=(t == 0),
                                stop=(t == 15),
                            )

                for t in range(16):
                    et_half = []
                    for half in range(2):
                        ps_sc = psc.tile([128, 1024], F32, tag="sc")
                        for sq in range(2):
                            nc.tensor.matmul(
                                ps_sc[:, 512 * sq : 512 * (sq + 1)],
                                lhsT=kT[u][:, 128 * t : 128 * (t + 1)],
                                rhs=qT[u][:, 1024 * half + 512 * sq : 1024 * half + 512 * (sq + 1)],
                                start=True,
                                stop=True,
                            )
                        if half == 1 and t % 2 == 0:
                            # offload ~1/4 of the exps to DVE via the
                            # exponent-field trick: bf16(int16(A*s + B)) ~=
                            # exp(s/8); softmax normalization cancels the
                            # systematic error (validated: <2e-4 effect on
                            # final rel err)
                            e16 = et_pool.tile(
                                [128, 1024], mybir.dt.uint16, tag="et",
                                name=f"e16_{u}_{t}",
                            )
                            nc.vector.tensor_scalar(
                                out=e16,
                                in0=ps_sc,
                                scalar1=0.125 * 128.0 / 0.6931471805599453,
                                scalar2=16256.0 - 6.5,
                                op0=ALU.mult,
                                op1=ALU.add,
                            )
                            et_half.append((e16, True))
                        else:
                            etb = et_pool.tile([128, 1024], BF16, tag="et")
                            nc.scalar.activation(
                                out=etb, in_=ps_sc, func=AF.Exp,
                                bias=0.0, scale=0.125,
                            )
                            et_half.append((etb, False))
                    if et_prev is not None:
                        emit_av(t - 1, et_prev)
                    et_prev = et_half
                emit_av(15, et_prev)

                # drain in 3 passes: recips, then normalizes (frees the PSUM
                # accumulators for the next consumer fast), then transposes.
                oT = ot_pool.tile([64, S], BF16, tag="oT", name=f"oT{u}")
                recs, o_ns = [], []
                for s_i in range(16):
                    g, jj = s_i // 4, s_i % 4
                    rec = small.tile(
                        [128, 1], F32, tag=f"rec{s_i}", name=f"rc{u}_{s_i}"
                    )
                    with nc.allow_low_precision(reason="softmax recip"):
                        nc.vector.reciprocal(
                            rec, o_ps[g][:, 65 * jj + 64 : 65 * jj + 65]
                        )
                    recs.append(rec)
                for s_i in range(16):
                    g, jj = s_i // 4, s_i % 4
                    o_n = small.tile(
                        [128, 64], BF16, tag=f"o_n{s_i}", name=f"on{u}_{s_i}"
                    )
                    nc.vector.tensor_scalar(
                        out=o_n,
                        in0=o_ps[g][:, 65 * jj : 65 * jj + 64],
                        scalar1=recs[s_i],
                        scalar2=None,
                        op0=ALU.mult,
                    )
                    o_ns.append(o_n)
                for s_i in range(16):
                    pt = psc.tile([64, 128], BF16, tag="sc", name=f"pt{u}_{s_i}")
                    nc.tensor.transpose(pt, o_ns[s_i], ident)
                    nc.vector.tensor_copy(oT[:, 128 * s_i : 128 * (s_i + 1)], pt)
                nc.sync.dma_start(
                    out=send_h[u][:].rearrange("(j p) c -> p j c", p=64),
                    in_=oT[:].rearrange("p (j c) -> p j c", j=8),
                )
                nc.gpsimd.collective_compute(
                    "AllToAll",
                    ALU.bypass,
                    replica_groups=GROUPS,
                    ins=[send_h[u][:].opt()],
                    outs=[recv_h[u][:].opt()],
                )

            # ---- token phase ----
            # unit A's blocks landed during unit B's attention; emitted after
            # unit B's send so they don't block it in the sync-queue FIFO
            ocA = [
                oc_pool.tile([64, HSH], BF16, tag=f"ocA{s}", name=f"ocA{s}")
                for s in range(8)
            ]
            for s in range(8):
                nc.sync.dma_start(
                    out=ocA[s], in_=recv_h[0][64 * s : 64 * (s + 1), :]
                )
            # Wo in two K=64 passes: pass 1 (even heads, from the first
            # exchange) fills the second AllToAll's latency window; pass 2
            # (odd heads) runs once the second exchange lands.
            ps_wo = [
                pacc.tile([128, 512], F32, tag="acc", name=f"pswo{i}")
                for i in range(4)
            ]
            last_p1 = None
            for i in range(4):
                bh, il = i // 2, i % 2
                for r in range(4):
                    last_p1 = nc.tensor.matmul(
                        ps_wo[i],
                        lhsT=ocA[4 * bh + r][:, 128 * il : 128 * (il + 1)],
                        rhs=woe_sb[:, 512 * r : 512 * (r + 1)],
                        start=(r == 0),
                        stop=False,
                    )
            # bridge the second AllToAll's latency so the FFN starts warm
            pe_warm(44, "a2a", pin_after=last_p1)
            ocB = [
                oc_pool.tile([64, HSH], BF16, tag=f"ocB{s}", name=f"ocB{s}")
                for s in range(8)
            ]
            for s in range(8):
                eng = nc.sync if s % 2 == 0 else nc.scalar
                eng.dma_start(out=ocB[s], in_=recv_h[1][64 * s : 64 * (s + 1), :])

            def ln_core(dst, src):
                # (x - mu) * rsqrt(var + eps); gamma/beta folded elsewhere
                st = small.tile([128, 6], F32, tag="st")
                nc.vector.bn_stats(st, src)
                mv = small.tile([128, 2], F32, tag="mv")
                nc.vector.bn_aggr(mv, st)
                rstd = small.tile([128, 1], F32, tag="rstd")
                nc.scalar.activation(
                    out=rstd, in_=mv[:, 1:2], func=AF.Sqrt, bias=eps_t, scale=1.0
                )
                nc.vector.reciprocal(rstd, rstd)
                nmr = small.tile([128, 1], F32, tag="nmr")
                nc.vector.tensor_scalar(
                    out=nmr,
                    in0=mv[:, 0:1],
                    scalar1=rstd,
                    scalar2=-1.0,
                    op0=ALU.mult,
                    op1=ALU.mult,
                )
                nc.scalar.activation(
                    out=dst, in_=src, func=AF.Identity, bias=nmr, scale=rstd
                )
                return rstd, nmr

            h_bf = [None] * 4
            hT = [
                h_pool.tile([128, 512], BF16, tag=f"hT{dd}", name=f"hT{dd}")
                for dd in range(4)
            ]
            for i in range(4):
                # token chunk i: chunks 0,1 = batch 0's 256 tokens; 2,3 = batch 1
                bh, il = i // 2, i % 2
                for r in range(4):
                    nc.tensor.matmul(
                        ps_wo[i],
                        lhsT=ocB[4 * bh + r][:, 128 * il : 128 * (il + 1)],
                        rhs=woo_sb[:, 512 * r : 512 * (r + 1)],
                        start=False,
                        stop=(r == 3),
                    )
                t1 = tmp_pool.tile([128, D], F32, tag="t1")
                nc.vector.tensor_tensor(out=t1, in0=ps_wo[i], in1=xsbo[i], op=ALU.add)
                hb = h_pool.tile([128, D], BF16, tag=f"h{i}", name=f"h{i}")
                ln_core(hb, t1)
                h_bf[i] = hb
                for dd in range(4):
                    pt = pacc.tile([128, 128], BF16, tag="acc", name=f"ph{i}_{dd}")
                    nc.tensor.transpose(pt, hb[:, 128 * dd : 128 * (dd + 1)], ident)
                    nc.vector.tensor_copy(hT[dd][:, 128 * i : 128 * (i + 1)], pt)

            ff_ps = [
                pacc.tile([128, 512], F32, tag="acc", name=f"ff{i}") for i in range(4)
            ]
            for f in range(16):
                ps1 = psc.tile([128, 512], F32, tag="sc", name=f"ps1_{f}")
                for dd in range(4):
                    nc.tensor.matmul(
                        ps1,
                        lhsT=w1_sb[:, 2048 * dd + 128 * f : 2048 * dd + 128 * (f + 1)],
                        rhs=hT[dd],
                        start=(dd == 0),
                        stop=(dd == 3),
                    )
                f1 = f1_pool.tile([128, 512], BF16, tag="f1", name=f"f1_{f}")
                nc.scalar.activation(
                    out=f1, in_=ps1, func=AF.Relu,
                    bias=cst_sb[:, 2 + f : 3 + f], scale=1.0,
                )
                for i in range(4):
                    nc.tensor.matmul(
                        ff_ps[i],
                        lhsT=f1[:, 128 * i : 128 * (i + 1)],
                        rhs=w2_sb[:, 512 * f : 512 * (f + 1)],
                        start=(f == 0),
                        stop=False,
                    )
            for i in range(4):
                nc.tensor.matmul(
                    ff_ps[i], lhsT=ones128, rhs=b2r, start=False, stop=True
                )

            for i in range(4):
                # residual 2: ff (+ b2 + ln1_b already accumulated) + h_core*g1
                hg = tmp_pool.tile([128, D], F32, tag="hg")
                nc.gpsimd.tensor_tensor(out=hg, in0=h_bf[i], in1=g1_t, op=ALU.mult)
                t2 = tmp_pool.tile([128, D], F32, tag="e2")
                nc.vector.tensor_tensor(out=t2, in0=ff_ps[i], in1=hg, op=ALU.add)
                tn = tmp_pool.tile([128, D], F32, tag="tn")
                ln_core(tn, t2)
                tg = tmp_pool.tile([128, D], F32, tag="tg")
                nc.vector.tensor_tensor(out=tg, in0=tn, in1=g2_t, op=ALU.mult)
                o_sb = tmp_pool.tile([128, D], F32, tag="osb")
                nc.gpsimd.tensor_tensor(out=o_sb, in0=tg, in1=be2_t, op=ALU.add)
                nc.sync.dma_start(out=out_h[128 * i : 128 * (i + 1), :], in_=o_sb)

    _split_excess_waits(nc)
    return nc


_NC_CACHE = {}


def _get_nc():
    if "nc" not in _NC_CACHE:
        _NC_CACHE["nc"] = build_nc()
    return _NC_CACHE["nc"]


def build_in_maps(inputs):
    bf = mybir.dt.np(BF16)
    x = np.asarray(inputs["x"], np.float32)
    Wq = np.asarray(inputs["Wq"], np.float32)
    bq = np.asarray(inputs["bq"], np.float32)
    Wk = np.asarray(inputs["Wk"], np.float32)
    bk = np.asarray(inputs["bk"], np.float32)
    Wv = np.asarray(inputs["Wv"], np.float32)
    bv = np.asarray(inputs["bv"], np.float32)
    Wo = np.asarray(inputs["Wo"], np.float32)
    bo = np.asarray(inputs["bo"], np.float32)
    ln1_g = np.asarray(inputs["ln1_g"], np.float32)
    ln1_b = np.asarray(inputs["ln1_b"], np.float32)
    W1 = np.asarray(inputs["W1"], np.float32)
    b1 = np.asarray(inputs["b1"], np.float32)
    W2 = np.asarray(inputs["W2"], np.float32)
    b2 = np.asarray(inputs["b2"], np.float32)
    ln2_g = np.asarray(inputs["ln2_g"], np.float32)
    ln2_b = np.asarray(inputs["ln2_b"], np.float32)

    def swz(w, p=128):
        # [K, N] -> [p, (K//p)*N] so SBUF partition lines are contiguous
        k, n = w.shape
        return np.ascontiguousarray(
            w.reshape(k // p, p, n).transpose(1, 0, 2).reshape(p, (k // p) * n)
        )

    # fold LN1 gamma into W1 rows and LN1 beta into the FFN bias / residual:
    #   relu((h*g1 + be1) @ W1 + b1) = relu(h @ (g1*W1) + (be1 @ W1 + b1))
    #   pre-LN2 sum gets + be1 via bb2 = b2 + be1 (ones-matmul path)
    w1s = swz(W1 * ln1_g[:, None]).astype(bf)
    b1f = b1 + ln1_b @ W1
    bb2 = (b2 + ln1_b).astype(np.float32)
    w2s = swz(W2).astype(bf)
    wo4 = Wo.reshape(4, 2, 64, D)
    woe = np.ascontiguousarray(
        wo4[:, 0].transpose(1, 0, 2).reshape(64, 4 * D)
    ).astype(bf)
    woo = np.ascontiguousarray(
        wo4[:, 1].transpose(1, 0, 2).reshape(64, 4 * D)
    ).astype(bf)
    xts = [np.ascontiguousarray(x[b].T).astype(bf) for b in range(B)]

    in_maps = []
    for c in range(N_CORES):
        b, r = c // 4, c % 4
        h0, h1 = 2 * r, 2 * r + 1
        wqk = swz(
            np.concatenate([Wq[h0], Wq[h1], Wk[h0], Wk[h1]], axis=1)
        ).astype(bf)
        wv2 = swz(np.concatenate([Wv[h0], Wv[h1]], axis=1)).astype(bf)
        cst = np.zeros((128, 18), np.float32)
        cst[0:64, 0] = bq[h0]
        cst[64:128, 0] = bq[h1]
        cst[0:64, 1] = bk[h0]
        cst[64:128, 1] = bk[h1]
        cst[:, 2:18] = b1f.reshape(16, 128).T
        bv2 = np.ascontiguousarray(
            np.broadcast_to(
                np.concatenate([bv[h0], bv[h1]])[None, :], (128, 128)
            )
        ).astype(np.float32)
        # token shard of core c: 256 tokens of batch 0 + 256 of batch 1
        xsb = (
            np.concatenate(
                [x[0, HSH * c : HSH * (c + 1)], x[1, HSH * c : HSH * (c + 1)]]
            )
            + bo[None, :]
        ).astype(bf)
        in_maps.append(
            {
                "xt": xts[b],
                "wqk": wqk,
                "wv": wv2,
                "woe": woe,
                "woo": woo,
                "w1": w1s,
                "w2": w2s,
                "xsb": xsb,
                "cst": cst,
                "bv": bv2,
                "g1": ln1_g,
                "g2": ln2_g,
                "be2": ln2_b,
                "b2": bb2,
            }
        )
    return in_maps


def kernel(**inputs):
    in_maps = build_in_maps(inputs)
    nc = _get_nc()
    res = run_bass_kernel_spmd(nc, in_maps, core_ids=list(range(N_CORES)))
    out = np.empty((B, S, D), np.float32)
    for c in range(N_CORES):
        sh = res.results[c]["out_shard"]
        out[0, HSH * c : HSH * (c + 1)] = sh[:HSH]
        out[1, HSH * c : HSH * (c + 1)] = sh[HSH:]
    return out


# revision 52
# speedup vs baseline: 1.1668x; 1.1668x over previous
"""Trainium2 Bass kernel for nn_EncoderLayer (D=512, H=8, DK=64, DF=2048, B=2, S=2048).

Strategy v2 (8 NeuronCores, batch-split tensor parallel):
  - Core c handles batch b=c//4 and heads (2*(c%4), 2*(c%4)+1). Groups of 4
    cores (one per die pair) cover one batch; the post-attention exchange is
    a 4-rank AllToAll within the group (same-die links, half the bytes).
  - All data bf16 (fp32 accumulation in PSUM); weights are pre-swizzled on
    the host into the exact SBUF layout so every DMA moves contiguous >=1KB
    lines in a handful of large transfers.
  - Attention: q/k kept feature-major (zero-padded to K=128); scores in
    PSUM -> exp on ACT -> AV matmul in token-major orientation
    (lhsT=exp-scores) so the softmax denominator lands as a per-partition
    column: reciprocal + tensor_scalar normalize are cheap, then a PE
    transpose produces the feature-major shard for the exchange.
  - Each head's output is shipped in its own AllToAll; the first one
    overlaps the second head's attention. After the exchange: Wo + LN1 +
    FFN + LN2 on this core's 512 tokens.
"""

import numpy as np

import concourse.bass as bass
import concourse.tile as tile
from concourse import mybir
from concourse.bass_utils import run_bass_kernel_spmd
from concourse.masks import make_identity

F32 = mybir.dt.float32
F32R = mybir.dt.float32r
BF16 = mybir.dt.bfloat16
AF = mybir.ActivationFunctionType
ALU = mybir.AluOpType

B, S, D, H, DK, DF = 2, 2048, 512, 8, 64, 2048
N_CORES = 8
NSH = 512  # tokens per core
EPS = 1e-5
GROUPS = [list(range(N_CORES))]
HSH = 256  # tokens per core per batch (shard = 256 of b0 + 256 of b1)

# ---------------------------------------------------------------------------
# Workaround: this walrus build rejects >1 sem wait on CTRL-type (drain)
# instructions. Split the TileContext tail-drain waits onto dedicated
# single-wait nops; the all-engine barrier right after keeps semantics.


def _split_excess_waits(nc, max_waits=1):
    for fn in nc.m.functions:
        for bb in fn.blocks:
            new_list = []
            for inst in bb.instructions:
                si = inst.sync_info
                waits = list(si.on_wait) if si is not None and si.on_wait else []
                if len(waits) > max_waits:
                    keep = waits[:max_waits]
                    extra = waits[max_waits:]
                    si.on_wait = keep
                    for w in extra:
                        nop = mybir.InstNoOp(name=f"I-waitnop-{nc.next_id()}")
                        nop.engine = inst.engine
                        nop.sync_info = mybir.SyncInfo(on_wait=[w], on_update=[])
                        new_list.append(nop)
                new_list.append(inst)
            bb.instructions = new_list


def _bcast_ap(handle, parts, n):
    """AP reading a 1-D DRAM tensor broadcast across `parts` partitions."""
    a = handle[:]
    return bass.AP(tensor=a.tensor, offset=a.offset, ap=[[0, parts], [1, n]])


def _v():
    import os

    return os.environ.get("KV2_VARIANT", "full")


def build_nc():
    nc = bass.Bass(target_bir_lowering=False)

    # ---- kernel I/O (per core; host pre-swizzles to SBUF layouts) ----
    xt_h = nc.dram_tensor("xt", [D, S], BF16, kind="ExternalInput")
    wqk_h = nc.dram_tensor("wqk", [128, 1024], BF16, kind="ExternalInput")
    wv_h = nc.dram_tensor("wv", [128, 512], BF16, kind="ExternalInput")
    # Wo split by even/odd head rows so both Wo passes use K=64 at offset 0
    woe_h = nc.dram_tensor("woe", [64, 2048], BF16, kind="ExternalInput")
    woo_h = nc.dram_tensor("woo", [64, 2048], BF16, kind="ExternalInput")
    w1_h = nc.dram_tensor("w1", [128, 8192], BF16, kind="ExternalInput")
    w2_h = nc.dram_tensor("w2", [128, 8192], BF16, kind="ExternalInput")
    xsb_h = nc.dram_tensor("xsb", [NSH, D], BF16, kind="ExternalInput")
    cst_h = nc.dram_tensor("cst", [128, 18], F32, kind="ExternalInput")
    bv_h = nc.dram_tensor("bv", [128, 128], F32, kind="ExternalInput")
    g1_h = nc.dram_tensor("g1", [D], F32, kind="ExternalInput")
    g2_h = nc.dram_tensor("g2", [D], F32, kind="ExternalInput")
    be2_h = nc.dram_tensor("be2", [D], F32, kind="ExternalInput")
    # bb2 = b2 + ln1_b (both added to the pre-LN2 sum via the ones matmul)
    b2_h = nc.dram_tensor("b2", [D], F32R, kind="ExternalInput")
    out_h = nc.dram_tensor("out_shard", [NSH, D], F32, kind="ExternalOutput")

    with tile.TileContext(nc) as tc:
        with (
            tc.tile_pool(name="consts", bufs=1) as consts,
            tc.tile_pool(name="qk", bufs=1) as qk_pool,
            tc.tile_pool(name="vaug", bufs=16) as v_pool,
            tc.tile_pool(name="et", bufs=5) as et_pool,
            tc.tile_pool(name="otn", bufs=2) as ot_pool,
            tc.tile_pool(name="oc", bufs=1) as oc_pool,
            tc.tile_pool(name="hh", bufs=1) as h_pool,
            tc.tile_pool(name="f1p", bufs=3) as f1_pool,
            tc.tile_pool(name="tmps", bufs=2) as tmp_pool,
            tc.tile_pool(name="small", bufs=4) as small,
            tc.tile_pool(name="psc", bufs=2, space="PSUM") as psc,
            tc.tile_pool(name="pacc", bufs=4, space="PSUM") as pacc,
            tc.tile_pool(name="dram", bufs=1, space="DRAM") as dram,
        ):
            # ---- input streams. sync queue: big loads in priority order;
            # gpsimd queue: small consts.
            wqk_sb = consts.tile([128, 1024], BF16, tag="wqk")
            nc.sync.dma_start(out=wqk_sb, in_=wqk_h[:, :])
            xt = []
            for dd in range(4):
                t_ = consts.tile([128, S], BF16, tag=f"xt{dd}")
                eng = nc.sync if dd % 2 == 0 else nc.scalar
                eng.dma_start(out=t_, in_=xt_h[128 * dd : 128 * (dd + 1), :])
                xt.append(t_)
            wv_sb = consts.tile([128, 512], BF16, tag="wv")
            nc.sync.dma_start(out=wv_sb, in_=wv_h[:, :])
            woe_sb = consts.tile([64, 2048], BF16, tag="woe")
            nc.sync.dma_start(out=woe_sb, in_=woe_h[:, :])
            woo_sb = consts.tile([64, 2048], BF16, tag="woo")
            nc.sync.dma_start(out=woo_sb, in_=woo_h[:, :])
            w1_sb = consts.tile([128, 8192], BF16, tag="w1")
            nc.sync.dma_start(out=w1_sb, in_=w1_h[:, :])
            w2_sb = consts.tile([128, 8192], BF16, tag="w2")
            nc.sync.dma_start(out=w2_sb, in_=w2_h[:, :])
            xsbo = []
            for i in range(4):
                t_ = consts.tile([128, D], BF16, tag=f"xsbo{i}")
                nc.sync.dma_start(out=t_, in_=xsb_h[128 * i : 128 * (i + 1), :])
                xsbo.append(t_)

            cst_sb = consts.tile([128, 18], F32, tag="cst")
            nc.gpsimd.dma_start(out=cst_sb, in_=cst_h[:, :])
            bv_sb = consts.tile([128, 128], F32, tag="bv")
            nc.gpsimd.dma_start(out=bv_sb, in_=bv_h[:, :])
            g1_t = consts.tile([128, D], F32, tag="g1_t")
            g2_t = consts.tile([128, D], F32, tag="g2_t")
            be2_t = consts.tile([128, D], F32, tag="be2_t")
            for t_sb, h_d in ((g1_t, g1_h), (g2_t, g2_h), (be2_t, be2_h)):
                nc.gpsimd.dma_start(out=t_sb, in_=_bcast_ap(h_d, 128, D))
            b2r = consts.tile([1, D], F32R, tag="b2r")
            nc.gpsimd.dma_start(out=b2r, in_=b2_h[:].rearrange("(o d) -> o d", o=1))

            ident = consts.tile([128, 128], BF16)
            make_identity(nc, ident)
            eps_t = consts.tile([128, 1], F32)
            nc.vector.memset(eps_t, EPS)
            ones128 = consts.tile([1, 128], F32R)
            nc.vector.memset(ones128[:].bitcast(F32), 1.0)
            warm_src = consts.tile([128, 512], BF16, tag="warm")
            nc.vector.memset(warm_src, 0.25)

            def pe_warm(n, name, pin_after=None):
                # dummy matmuls to hold the PE HAM clock-gate open across
                # windows where real matmul work is briefly unavailable
                wp = psc.tile([128, 512], F32, tag="sc", name=f"warm_{name}")
                first = None
                for k in range(n):
                    mm = nc.tensor.matmul(
                        wp,
                        lhsT=warm_src[:, 0:128],
                        rhs=warm_src,
                        start=True,
                        stop=True,
                    )
                    if first is None:
                        first = mm
                if pin_after is not None and first is not None:
                    tile.add_dep_helper(
                        first.ins, pin_after.ins, sync=True,
                        reason="keep PE warm only after the preceding block",
                    )

            pe_warm(18, "boot")

            # ---- QKV projections for both heads of this core ----
            # qT_u/kT_u: [128, S] bf16, rows 0-63 = head u's projection,
            # rows 64-127 zeroed so score matmuls run with K=128.
            qT = [
                qk_pool.tile([128, S], BF16, tag=f"qT{u}", name=f"qT{u}")
                for u in range(2)
            ]
            kT = [
                qk_pool.tile([128, S], BF16, tag=f"kT{u}", name=f"kT{u}")
                for u in range(2)
            ]
            for t_ in qT + kT:
                nc.gpsimd.memset(t_[64:128, :], 0.0)

            for s4 in range(4):
                for qk, dst, bcol in ((0, qT, 0), (1, kT, 1)):
                    ps = psc.tile([128, 512], F32, tag="sc")
                    for dd in range(4):
                        nc.tensor.matmul(
                            ps,
                            lhsT=wqk_sb[:, 256 * dd + 128 * qk : 256 * dd + 128 * (qk + 1)],
                            rhs=xt[dd][:, 512 * s4 : 512 * (s4 + 1)],
                            start=(dd == 0),
                            stop=(dd == 3),
                        )
                    # head 0 copy+bias on DVE, head 1 on ACT — parallel engines
                    nc.vector.tensor_scalar_add(
                        dst[0][0:64, 512 * s4 : 512 * (s4 + 1)],
                        ps[0:64, :],
                        cst_sb[0:64, bcol : bcol + 1],
                    )
                    nc.scalar.activation(
                        out=dst[1][0:64, 512 * s4 : 512 * (s4 + 1)],
                        in_=ps[64:128, :],
                        func=AF.Identity,
                        bias=cst_sb[64:128, bcol : bcol + 1],
                        scale=1.0,
                    )

            # v_aug[t]: [128 tokens, 130] = [v_h0 (64) | 1 | v_h1 (64) | 1]
            v_aug = []
            for t in range(16):
                va = v_pool.tile([128, 130], BF16, tag="vaug", name=f"va{t}")
                va_v = va[:].rearrange("p (u c) -> p u c", c=65)
                nc.gpsimd.memset(va_v[:, :, 64:65], 1.0)
                psv = pacc.tile([128, 128], F32, tag="acc", name=f"psv{t}")
                for dd in range(4):
                    nc.tensor.matmul(
                        psv,
                        lhsT=xt[dd][:, 128 * t : 128 * (t + 1)],
                        rhs=wv_sb[:, 128 * dd : 128 * (dd + 1)],
                        start=(dd == 0),
                        stop=(dd == 3),
                    )
                nc.vector.tensor_tensor(
                    out=va_v[:, :, 0:64],
                    in0=psv[:].rearrange("p (u c) -> p u c", c=64),
                    in1=bv_sb[:].rearrange("p (u c) -> p u c", c=64),
                    op=ALU.add,
                )
                v_aug.append(va)

            # per-unit exchange buffers: 8 blocks of [128 tokens, 128
            # (2 sub-chunks x 64 feats)] in token-major layout
            send_h = [dram.tile([1024, 128], BF16, name=f"send{u}") for u in range(2)]
            recv_h = [dram.tile([1024, 128], BF16, name=f"recv{u}") for u in range(2)]

            # ---- attention per head-unit ----
            for u in range(2):
                # o accumulators: 4 PSUM tiles, each holds 4 s-chunks x 65
                # (64 v-cols + denominator from the ones column).
                o_ps = [
                    pacc.tile([128, 260], F32, tag="acc", name=f"ops{u}_{g}")
                    for g in range(4)
                ]
                et_prev = None

                def emit_av(t, et_half):
                    for half in range(2):
                        et_t, is_u16 = et_half[half]
                        for sl in range(8):
                            s_i = 8 * half + sl
                            lhsT = et_t[:, 128 * sl : 128 * (sl + 1)]
                            if is_u16:
                                lhsT = lhsT.bitcast(BF16)
                            nc.tensor.matmul(
                                o_ps[s_i // 4][:, 65 * (s_i % 4) : 65 * (s_i % 4) + 65],
                                lhsT=lhsT,
                                rhs=v_aug[t][:, 65 * u : 65 * (u + 1)],
                                start=(t == 0),
                                stop=(t == 15),
                            )

                for t in range(16):
                    et_half = []
                    for half in range(2):
                        ps_sc = psc.tile([128, 1024], F32, tag="sc")
                        for sq in range(2):
                            nc.tensor.matmul(
                                ps_sc[:, 512 * sq : 512 * (sq + 1)],
                                lhsT=kT[u][:, 128 * t : 128 * (t + 1)],
                                rhs=qT[u][:, 1024 * half + 512 * sq : 1024 * half + 512 * (sq + 1)],
                                start=True,
                                stop=True,
                            )
                        if half == 1 and t % 2 == 0:
                            # offload ~1/4 of the exps to DVE via the
                            # exponent-field trick: bf16(int16(A*s + B)) ~=
                            # exp(s/8); softmax normalization cancels the
                            # systematic error (validated: <2e-4 effect on
                            # final rel err)
                            e16 = et_pool.tile(
                                [128, 1024], mybir.dt.uint16, tag="et",
                                name=f"e16_{u}_{t}",
                            )
                            nc.vector.tensor_scalar(
                                out=e16,
                                in0=ps_sc,
                                scalar1=0.125 * 128.0 / 0.6931471805599453,
                                scalar2=16256.0 - 6.5,
                                op0=ALU.mult,
                                op1=ALU.add,
                            )
                            et_half.append((e16, True))
                        else:
                            etb = et_pool.tile([128, 1024], BF16, tag="et")
                            nc.scalar.activation(
                                out=etb, in_=ps_sc, func=AF.Exp,
                                bias=0.0, scale=0.125,
                            )
                            et_half.append((etb, False))
                    if et_prev is not None:
                        emit_av(t - 1, et_prev)
                    et_prev = et_half
                emit_av(15, et_prev)

                # drain: recips first, then normalize straight into the
                # token-major ship tile (frees PSUM fast; no transposes here
                # - the receiver transposes during the exchange window)
                o_tok = ot_pool.tile([128, 1024], BF16, tag="oT", name=f"oT{u}")
                recs = []
                for s_i in range(16):
                    g, jj = s_i // 4, s_i % 4
                    rec = small.tile(
                        [128, 1], F32, tag=f"rec{s_i}", name=f"rc{u}_{s_i}"
                    )
                    with nc.allow_low_precision(reason="softmax recip"):
                        nc.vector.reciprocal(
                            rec, o_ps[g][:, 65 * jj + 64 : 65 * jj + 65]
                        )
                    recs.append(rec)
                for s_i in range(16):
                    g, jj = s_i // 4, s_i % 4
                    nc.vector.tensor_scalar(
                        out=o_tok[:, 64 * s_i : 64 * (s_i + 1)],
                        in0=o_ps[g][:, 65 * jj : 65 * jj + 64],
                        scalar1=recs[s_i],
                        scalar2=None,
                        op0=ALU.mult,
                    )
                nc.sync.dma_start(
                    out=send_h[u][:].rearrange("(j p) c -> p j c", p=128),
                    in_=o_tok[:].rearrange("p (j c) -> p j c", j=8),
                )
                nc.gpsimd.collective_compute(
                    "AllToAll",
                    ALU.bypass,
                    replica_groups=GROUPS,
                    ins=[send_h[u][:].opt()],
                    outs=[recv_h[u][:].opt()],
                )

            # ---- token phase ----
            # unit A's blocks landed during unit B's attention; loads and
            # transposes are emitted after unit B so their pool-slot requests
            # queue behind unit B's and naturally fill the exchange window
            rtA = oc_pool.tile([128, 1024], BF16, tag="rtA", name="rtA")
            nc.sync.dma_start(
                out=rtA[:].rearrange("p (j c) -> p j c", c=128),
                in_=recv_h[0][:].rearrange("(j p) c -> p j c", p=128)
            )
            ocA = [
                oc_pool.tile([64, HSH], BF16, tag=f"ocA{s}", name=f"ocA{s}")
                for s in range(8)
            ]
            for s in range(8):
                for hf in range(2):
                    pt = psc.tile([64, 128], BF16, tag="sc", name=f"ptA{s}_{hf}")
                    nc.tensor.transpose(
                        pt, rtA[:, 128 * s + 64 * hf : 128 * s + 64 * (hf + 1)], ident
                    )
                    nc.vector.tensor_copy(
                        ocA[s][:, 128 * hf : 128 * (hf + 1)], pt
                    )
            # Wo in two K=64 passes: pass 1 (even heads, from the first
            # exchange) fills the second AllToAll's latency window; pass 2
            # (odd heads) runs once the second exchange lands.
            ps_wo = [
                pacc.tile([128, 512], F32, tag="acc", name=f"pswo{i}")
                for i in range(4)
            ]
            last_p1 = None
            for i in range(4):
                bh, il = i // 2, i % 2
                for r in range(4):
                    last_p1 = nc.tensor.matmul(
                        ps_wo[i],
                        lhsT=ocA[4 * bh + r][:, 128 * il : 128 * (il + 1)],
                        rhs=woe_sb[:, 512 * r : 512 * (r + 1)],
                        start=(r == 0),
                        stop=False,
                    )
            # bridge the second AllToAll's latency so the FFN starts warm
            pe_warm(44, "a2a", pin_after=last_p1)
            rtB = oc_pool.tile([128, 1024], BF16, tag="rtB", name="rtB")
            nc.scalar.dma_start(
                out=rtB[:].rearrange("p (j c) -> p j c", c=128),
                in_=recv_h[1][:].rearrange("(j p) c -> p j c", p=128)
            )
            ocB = [
                oc_pool.tile([64, HSH], BF16, tag=f"ocB{s}", name=f"ocB{s}")
                for s in range(8)
            ]
            for s in range(8):
                for hf in range(2):
                    pt = psc.tile([64, 128], BF16, tag="sc", name=f"ptB{s}_{hf}")
                    nc.tensor.transpose(
                        pt, rtB[:, 128 * s + 64 * hf : 128 * s + 64 * (hf + 1)], ident
                    )
                    nc.vector.tensor_copy(
                        ocB[s][:, 128 * hf : 128 * (hf + 1)], pt
                    )

            def ln_core(dst, src):
                # (x - mu) * rsqrt(var + eps); gamma/beta folded elsewhere
                st = small.tile([128, 6], F32, tag="st")
                nc.vector.bn_stats(st, src)
                mv = small.tile([128, 2], F32, tag="mv")
                nc.vector.bn_aggr(mv, st)
                rstd = small.tile([128, 1], F32, tag="rstd")
                nc.scalar.activation(
                    out=rstd, in_=mv[:, 1:2], func=AF.Sqrt, bias=eps_t, scale=1.0
                )
                nc.vector.reciprocal(rstd, rstd)
                nmr = small.tile([128, 1], F32, tag="nmr")
                nc.vector.tensor_scalar(
                    out=nmr,
                    in0=mv[:, 0:1],
                    scalar1=rstd,
                    scalar2=-1.0,
                    op0=ALU.mult,
                    op1=ALU.mult,
                )
                nc.scalar.activation(
                    out=dst, in_=src, func=AF.Identity, bias=nmr, scale=rstd
                )
                return rstd, nmr

            h_bf = [None] * 4
            hT = [
                h_pool.tile([128, 512], BF16, tag=f"hT{dd}", name=f"hT{dd}")
                for dd in range(4)
            ]
            for i in range(4):
                # token chunk i: chunks 0,1 = batch 0's 256 tokens; 2,3 = batch 1
                bh, il = i // 2, i % 2
                for r in range(4):
                    nc.tensor.matmul(
                        ps_wo[i],
                        lhsT=ocB[4 * bh + r][:, 128 * il : 128 * (il + 1)],
                        rhs=woo_sb[:, 512 * r : 512 * (r + 1)],
                        start=False,
                        stop=(r == 3),
                    )
                t1 = tmp_pool.tile([128, D], F32, tag="t1")
                nc.vector.tensor_tensor(out=t1, in0=ps_wo[i], in1=xsbo[i], op=ALU.add)
                hb = h_pool.tile([128, D], BF16, tag=f"h{i}", name=f"h{i}")
                ln_core(hb, t1)
                h_bf[i] = hb
                for dd in range(4):
                    pt = pacc.tile([128, 128], BF16, tag="acc", name=f"ph{i}_{dd}")
                    nc.tensor.transpose(pt, hb[:, 128 * dd : 128 * (dd + 1)], ident)
                    nc.vector.tensor_copy(hT[dd][:, 128 * i : 128 * (i + 1)], pt)

            ff_ps = [
                pacc.tile([128, 512], F32, tag="acc", name=f"ff{i}") for i in range(4)
            ]
            for f in range(16):
                ps1 = psc.tile([128, 512], F32, tag="sc", name=f"ps1_{f}")
                for dd in range(4):
                    nc.tensor.matmul(
                        ps1,
                        lhsT=w1_sb[:, 2048 * dd + 128 * f : 2048 * dd + 128 * (f + 1)],
                        rhs=hT[dd],
                        start=(dd == 0),
                        stop=(dd == 3),
                    )
                f1 = f1_pool.tile([128, 512], BF16, tag="f1", name=f"f1_{f}")
                nc.scalar.activation(
                    out=f1, in_=ps1, func=AF.Relu,
                    bias=cst_sb[:, 2 + f : 3 + f], scale=1.0,
                )
                for i in range(4):
                    nc.tensor.matmul(
                        ff_ps[i],
                        lhsT=f1[:, 128 * i : 128 * (i + 1)],
                        rhs=w2_sb[:, 512 * f : 512 * (f + 1)],
                        start=(f == 0),
                        stop=False,
                    )
            for i in range(4):
                nc.tensor.matmul(
                    ff_ps[i], lhsT=ones128, rhs=b2r, start=False, stop=True
                )

            for i in range(4):
                # residual 2: ff (+ b2 + ln1_b already accumulated) + h_core*g1
                hg = tmp_pool.tile([128, D], F32, tag="hg")
                nc.gpsimd.tensor_tensor(out=hg, in0=h_bf[i], in1=g1_t, op=ALU.mult)
                t2 = tmp_pool.tile([128, D], F32, tag="e2")
                nc.vector.tensor_tensor(out=t2, in0=ff_ps[i], in1=hg, op=ALU.add)
                tn = tmp_pool.tile([128, D], F32, tag="tn")
                ln_core(tn, t2)
                tg = tmp_pool.tile([128, D], F32, tag="tg")
                nc.vector.tensor_tensor(out=tg, in0=tn, in1=g2_t, op=ALU.mult)
                o_sb = tmp_pool.tile([128, D], F32, tag="osb")
                nc.gpsimd.tensor_tensor(out=o_sb, in0=tg, in1=be2_t, op=ALU.add)
                nc.sync.dma_start(out=out_h[128 * i : 128 * (i + 1), :], in_=o_sb)

    _split_excess_waits(nc)
    return nc


_NC_CACHE = {}


def _get_nc():
    if "nc" not in _NC_CACHE:
        _NC_CACHE["nc"] = build_nc()
    return _NC_CACHE["nc"]


def build_in_maps(inputs):
    bf = mybir.dt.np(BF16)
    x = np.asarray(inputs["x"], np.float32)
    Wq = np.asarray(inputs["Wq"], np.float32)
    bq = np.asarray(inputs["bq"], np.float32)
    Wk = np.asarray(inputs["Wk"], np.float32)
    bk = np.asarray(inputs["bk"], np.float32)
    Wv = np.asarray(inputs["Wv"], np.float32)
    bv = np.asarray(inputs["bv"], np.float32)
    Wo = np.asarray(inputs["Wo"], np.float32)
    bo = np.asarray(inputs["bo"], np.float32)
    ln1_g = np.asarray(inputs["ln1_g"], np.float32)
    ln1_b = np.asarray(inputs["ln1_b"], np.float32)
    W1 = np.asarray(inputs["W1"], np.float32)
    b1 = np.asarray(inputs["b1"], np.float32)
    W2 = np.asarray(inputs["W2"], np.float32)
    b2 = np.asarray(inputs["b2"], np.float32)
    ln2_g = np.asarray(inputs["ln2_g"], np.float32)
    ln2_b = np.asarray(inputs["ln2_b"], np.float32)

    def swz(w, p=128):
        # [K, N] -> [p, (K//p)*N] so SBUF partition lines are contiguous
        k, n = w.shape
        return np.ascontiguousarray(
            w.reshape(k // p, p, n).transpose(1, 0, 2).reshape(p, (k // p) * n)
        )

    # fold LN1 gamma into W1 rows and LN1 beta into the FFN bias / residual:
    #   relu((h*g1 + be1) @ W1 + b1) = relu(h @ (g1*W1) + (be1 @ W1 + b1))
    #   pre-LN2 sum gets + be1 via bb2 = b2 + be1 (ones-matmul path)
    w1s = swz(W1 * ln1_g[:, None]).astype(bf)
    b1f = b1 + ln1_b @ W1
    bb2 = (b2 + ln1_b).astype(np.float32)
    w2s = swz(W2).astype(bf)
    wo4 = Wo.reshape(4, 2, 64, D)
    woe = np.ascontiguousarray(
        wo4[:, 0].transpose(1, 0, 2).reshape(64, 4 * D)
    ).astype(bf)
    woo = np.ascontiguousarray(
        wo4[:, 1].transpose(1, 0, 2).reshape(64, 4 * D)
    ).astype(bf)
    xts = [np.ascontiguousarray(x[b].T).astype(bf) for b in range(B)]

    in_maps = []
    for c in range(N_CORES):
        b, r = c // 4, c % 4
        h0, h1 = 2 * r, 2 * r + 1
        wqk = swz(
            np.concatenate([Wq[h0], Wq[h1], Wk[h0], Wk[h1]], axis=1)
        ).astype(bf)
        wv2 = swz(np.concatenate([Wv[h0], Wv[h1]], axis=1)).astype(bf)
        cst = np.zeros((128, 18), np.float32)
        cst[0:64, 0] = bq[h0]
        cst[64:128, 0] = bq[h1]
        cst[0:64, 1] = bk[h0]
        cst[64:128, 1] = bk[h1]
        cst[:, 2:18] = b1f.reshape(16, 128).T
        bv2 = np.ascontiguousarray(
            np.broadcast_to(
                np.concatenate([bv[h0], bv[h1]])[None, :], (128, 128)
            )
        ).astype(np.float32)
        # token shard of core c: 256 tokens of batch 0 + 256 of batch 1
        xsb = (
            np.concatenate(
                [x[0, HSH * c : HSH * (c + 1)], x[1, HSH * c : HSH * (c + 1)]]
            )
            + bo[None, :]
        ).astype(bf)
        in_maps.append(
            {
                "xt": xts[b],
                "wqk": wqk,
                "wv": wv2,
                "woe": woe,
                "woo": woo,
                "w1": w1s,
                "w2": w2s,
                "xsb": xsb,
                "cst": cst,
                "bv": bv2,
                "g1": ln1_g,
                "g2": ln2_g,
                "be2": ln2_b,
                "b2": bb2,
            }
        )
    return in_maps


def kernel(**inputs):
    in_maps = build_in_maps(inputs)
    nc = _get_nc()
    res = run_bass_kernel_spmd(nc, in_maps, core_ids=list(range(N_CORES)))
    out = np.empty((B, S, D), np.float32)
    for c in range(N_CORES):
        sh = res.results[c]["out_shard"]
        out[0, HSH * c : HSH * (c + 1)] = sh[:HSH]
        out[1, HSH * c : HSH * (c + 1)] = sh[HSH:]
    return out


# revision 58
# speedup vs baseline: 1.1830x; 1.0138x over previous
"""Trainium2 Bass kernel for nn_EncoderLayer (D=512, H=8, DK=64, DF=2048, B=2, S=2048).

Strategy v2 (8 NeuronCores, batch-split tensor parallel):
  - Core c handles batch b=c//4 and heads (2*(c%4), 2*(c%4)+1). Groups of 4
    cores (one per die pair) cover one batch; the post-attention exchange is
    a 4-rank AllToAll within the group (same-die links, half the bytes).
  - All data bf16 (fp32 accumulation in PSUM); weights are pre-swizzled on
    the host into the exact SBUF layout so every DMA moves contiguous >=1KB
    lines in a handful of large transfers.
  - Attention: q/k kept feature-major (zero-padded to K=128); scores in
    PSUM -> exp on ACT -> AV matmul in token-major orientation
    (lhsT=exp-scores) so the softmax denominator lands as a per-partition
    column: reciprocal + tensor_scalar normalize are cheap, then a PE
    transpose produces the feature-major shard for the exchange.
  - Each head's output is shipped in its own AllToAll; the first one
    overlaps the second head's attention. After the exchange: Wo + LN1 +
    FFN + LN2 on this core's 512 tokens.
"""

import numpy as np

import concourse.bass as bass
import concourse.tile as tile
from concourse import mybir
from concourse.bass_utils import run_bass_kernel_spmd
from concourse.masks import make_identity

F32 = mybir.dt.float32
F32R = mybir.dt.float32r
BF16 = mybir.dt.bfloat16
AF = mybir.ActivationFunctionType
ALU = mybir.AluOpType

B, S, D, H, DK, DF = 2, 2048, 512, 8, 64, 2048
N_CORES = 8
NSH = 512  # tokens per core
EPS = 1e-5
GROUPS = [list(range(N_CORES))]
HSH = 256  # tokens per core per batch (shard = 256 of b0 + 256 of b1)

# ---------------------------------------------------------------------------
# Workaround: this walrus build rejects >1 sem wait on CTRL-type (drain)
# instructions. Split the TileContext tail-drain waits onto dedicated
# single-wait nops; the all-engine barrier right after keeps semantics.


def _split_excess_waits(nc, max_waits=1):
    for fn in nc.m.functions:
        for bb in fn.blocks:
            new_list = []
            for inst in bb.instructions:
                si = inst.sync_info
                waits = list(si.on_wait) if si is not None and si.on_wait else []
                if len(waits) > max_waits:
                    keep = waits[:max_waits]
                    extra = waits[max_waits:]
                    si.on_wait = keep
                    for w in extra:
                        nop = mybir.InstNoOp(name=f"I-waitnop-{nc.next_id()}")
                        nop.engine = inst.engine
                        nop.sync_info = mybir.SyncInfo(on_wait=[w], on_update=[])
                        new_list.append(nop)
                new_list.append(inst)
            bb.instructions = new_list


def _bcast_ap(handle, parts, n):
    """AP reading a 1-D DRAM tensor broadcast across `parts` partitions."""
    a = handle[:]
    return bass.AP(tensor=a.tensor, offset=a.offset, ap=[[0, parts], [1, n]])


def _v():
    import os

    return os.environ.get("KV2_VARIANT", "full")


def build_nc():
    nc = bass.Bass(target_bir_lowering=False)

    # ---- kernel I/O (per core; host pre-swizzles to SBUF layouts) ----
    xt_h = nc.dram_tensor("xt", [D, S], BF16, kind="ExternalInput")
    wqk_h = nc.dram_tensor("wqk", [128, 1024], BF16, kind="ExternalInput")
    wv_h = nc.dram_tensor("wv", [128, 512], BF16, kind="ExternalInput")
    # Wo split by even/odd head rows so both Wo passes use K=64 at offset 0
    woe_h = nc.dram_tensor("woe", [64, 2048], BF16, kind="ExternalInput")
    woo_h = nc.dram_tensor("woo", [64, 2048], BF16, kind="ExternalInput")
    w1_h = nc.dram_tensor("w1", [128, 8192], BF16, kind="ExternalInput")
    w2_h = nc.dram_tensor("w2", [128, 8192], BF16, kind="ExternalInput")
    xsb_h = nc.dram_tensor("xsb", [NSH, D], BF16, kind="ExternalInput")
    cst_h = nc.dram_tensor("cst", [128, 18], F32, kind="ExternalInput")
    bv_h = nc.dram_tensor("bv", [128, 128], F32, kind="ExternalInput")
    g1_h = nc.dram_tensor("g1", [D], F32, kind="ExternalInput")
    g2_h = nc.dram_tensor("g2", [D], F32, kind="ExternalInput")
    be2_h = nc.dram_tensor("be2", [D], F32, kind="ExternalInput")
    # bb2 = b2 + ln1_b (both added to the pre-LN2 sum via the ones matmul)
    b2_h = nc.dram_tensor("b2", [D], F32R, kind="ExternalInput")
    out_h = nc.dram_tensor("out_shard", [NSH, D], F32, kind="ExternalOutput")

    with tile.TileContext(nc) as tc:
        with (
            tc.tile_pool(name="consts", bufs=1) as consts,
            tc.tile_pool(name="qk", bufs=1) as qk_pool,
            tc.tile_pool(name="vaug", bufs=16) as v_pool,
            tc.tile_pool(name="et", bufs=5) as et_pool,
            tc.tile_pool(name="otn", bufs=2) as ot_pool,
            tc.tile_pool(name="oc", bufs=1) as oc_pool,
            tc.tile_pool(name="hh", bufs=1) as h_pool,
            tc.tile_pool(name="f1p", bufs=3) as f1_pool,
            tc.tile_pool(name="tmps", bufs=2) as tmp_pool,
            tc.tile_pool(name="small", bufs=4) as small,
            tc.tile_pool(name="psc", bufs=2, space="PSUM") as psc,
            tc.tile_pool(name="pacc", bufs=4, space="PSUM") as pacc,
            tc.tile_pool(name="dram", bufs=1, space="DRAM") as dram,
        ):
            # ---- input streams. sync queue: big loads in priority order;
            # gpsimd queue: small consts.
            wqk_sb = consts.tile([128, 1024], BF16, tag="wqk")
            nc.sync.dma_start(out=wqk_sb, in_=wqk_h[:, :])
            xt = []
            for dd in range(4):
                t_ = consts.tile([128, S], BF16, tag=f"xt{dd}")
                eng = nc.sync if dd % 2 == 0 else nc.scalar
                eng.dma_start(out=t_, in_=xt_h[128 * dd : 128 * (dd + 1), :])
                xt.append(t_)
            wv_sb = consts.tile([128, 512], BF16, tag="wv")
            nc.sync.dma_start(out=wv_sb, in_=wv_h[:, :])
            woe_sb = consts.tile([64, 2048], BF16, tag="woe")
            nc.sync.dma_start(out=woe_sb, in_=woe_h[:, :])
            woo_sb = consts.tile([64, 2048], BF16, tag="woo")
            nc.sync.dma_start(out=woo_sb, in_=woo_h[:, :])
            w1_sb = consts.tile([128, 8192], BF16, tag="w1")
            nc.sync.dma_start(out=w1_sb, in_=w1_h[:, :])
            w2_sb = consts.tile([128, 8192], BF16, tag="w2")
            nc.sync.dma_start(out=w2_sb, in_=w2_h[:, :])
            xsbo = []
            for i in range(4):
                t_ = consts.tile([128, D], BF16, tag=f"xsbo{i}")
                nc.sync.dma_start(out=t_, in_=xsb_h[128 * i : 128 * (i + 1), :])
                xsbo.append(t_)

            cst_sb = consts.tile([128, 18], F32, tag="cst")
            nc.gpsimd.dma_start(out=cst_sb, in_=cst_h[:, :])
            bv_sb = consts.tile([128, 128], F32, tag="bv")
            nc.gpsimd.dma_start(out=bv_sb, in_=bv_h[:, :])
            g1_t = consts.tile([128, D], F32, tag="g1_t")
            g2_t = consts.tile([128, D], F32, tag="g2_t")
            be2_t = consts.tile([128, D], F32, tag="be2_t")
            for t_sb, h_d in ((g1_t, g1_h), (g2_t, g2_h), (be2_t, be2_h)):
                nc.gpsimd.dma_start(out=t_sb, in_=_bcast_ap(h_d, 128, D))
            b2r = consts.tile([1, D], F32R, tag="b2r")
            nc.gpsimd.dma_start(out=b2r, in_=b2_h[:].rearrange("(o d) -> o d", o=1))

            ident = consts.tile([128, 128], BF16)
            make_identity(nc, ident)
            eps_t = consts.tile([128, 1], F32)
            nc.vector.memset(eps_t, EPS)
            ones128 = consts.tile([1, 128], F32R)
            nc.vector.memset(ones128[:].bitcast(F32), 1.0)
            warm_src = consts.tile([128, 512], BF16, tag="warm")
            nc.vector.memset(warm_src, 0.25)

            def pe_warm(n, name, pin_after=None):
                # dummy matmuls to hold the PE HAM clock-gate open across
                # windows where real matmul work is briefly unavailable
                wp = psc.tile([128, 512], F32, tag="sc", name=f"warm_{name}")
                first = None
                for k in range(n):
                    mm = nc.tensor.matmul(
                        wp,
                        lhsT=warm_src[:, 0:128],
                        rhs=warm_src,
                        start=True,
                        stop=True,
                    )
                    if first is None:
                        first = mm
                if pin_after is not None and first is not None:
                    tile.add_dep_helper(
                        first.ins, pin_after.ins, sync=True,
                        reason="keep PE warm only after the preceding block",
                    )

            pe_warm(18, "boot")

            # ---- QKV projections for both heads of this core ----
            # qT_u/kT_u: [128, S] bf16, rows 0-63 = head u's projection,
            # rows 64-127 zeroed so score matmuls run with K=128.
            qT = [
                qk_pool.tile([128, S], BF16, tag=f"qT{u}", name=f"qT{u}")
                for u in range(2)
            ]
            kT = [
                qk_pool.tile([128, S], BF16, tag=f"kT{u}", name=f"kT{u}")
                for u in range(2)
            ]
            for t_ in qT + kT:
                nc.gpsimd.memset(t_[64:128, :], 0.0)

            for s4 in range(4):
                for qk, dst, bcol in ((0, qT, 0), (1, kT, 1)):
                    ps = psc.tile([128, 512], F32, tag="sc")
                    for dd in range(4):
                        nc.tensor.matmul(
                            ps,
                            lhsT=wqk_sb[:, 256 * dd + 128 * qk : 256 * dd + 128 * (qk + 1)],
                            rhs=xt[dd][:, 512 * s4 : 512 * (s4 + 1)],
                            start=(dd == 0),
                            stop=(dd == 3),
                        )
                    # head 0 copy+bias on DVE, head 1 on ACT — parallel engines
                    nc.vector.tensor_scalar_add(
                        dst[0][0:64, 512 * s4 : 512 * (s4 + 1)],
                        ps[0:64, :],
                        cst_sb[0:64, bcol : bcol + 1],
                    )
                    nc.scalar.activation(
                        out=dst[1][0:64, 512 * s4 : 512 * (s4 + 1)],
                        in_=ps[64:128, :],
                        func=AF.Identity,
                        bias=cst_sb[64:128, bcol : bcol + 1],
                        scale=1.0,
                    )

            # v_aug[t]: [128 tokens, 130] = [v_h0 (64) | 1 | v_h1 (64) | 1]
            v_aug = []
            last_v = None
            for t in range(16):
                va = v_pool.tile([128, 130], BF16, tag="vaug", name=f"va{t}")
                va_v = va[:].rearrange("p (u c) -> p u c", c=65)
                nc.gpsimd.memset(va_v[:, :, 64:65], 1.0)
                psv = pacc.tile([128, 128], F32, tag="acc", name=f"psv{t}")
                for dd in range(4):
                    last_v = nc.tensor.matmul(
                        psv,
                        lhsT=xt[dd][:, 128 * t : 128 * (t + 1)],
                        rhs=wv_sb[:, 128 * dd : 128 * (dd + 1)],
                        start=(dd == 0),
                        stop=(dd == 3),
                    )
                nc.vector.tensor_tensor(
                    out=va_v[:, :, 0:64],
                    in0=psv[:].rearrange("p (u c) -> p u c", c=64),
                    in1=bv_sb[:].rearrange("p (u c) -> p u c", c=64),
                    op=ALU.add,
                )
                v_aug.append(va)
            # bridge the gap between the V projections and the first scores
            # (q/k bias copies pace the start of the attention loop)
            pe_warm(14, "qkv", pin_after=last_v)

            # per-unit exchange buffers: 8 blocks of [128 tokens, 128
            # (2 sub-chunks x 64 feats)] in token-major layout
            send_h = [dram.tile([1024, 128], BF16, name=f"send{u}") for u in range(2)]
            recv_h = [dram.tile([1024, 128], BF16, name=f"recv{u}") for u in range(2)]

            # ---- attention per head-unit ----
            for u in range(2):
                # o accumulators: 4 PSUM tiles, each holds 4 s-chunks x 65
                # (64 v-cols + denominator from the ones column).
                o_ps = [
                    pacc.tile([128, 260], F32, tag="acc", name=f"ops{u}_{g}")
                    for g in range(4)
                ]
                et_prev = None

                def emit_av(t, et_half):
                    for half in range(2):
                        et_t, is_u16 = et_half[half]
                        for sl in range(8):
                            s_i = 8 * half + sl
                            lhsT = et_t[:, 128 * sl : 128 * (sl + 1)]
                            if is_u16:
                                lhsT = lhsT.bitcast(BF16)
                            nc.tensor.matmul(
                                o_ps[s_i // 4][:, 65 * (s_i % 4) : 65 * (s_i % 4) + 65],
                                lhsT=lhsT,
                                rhs=v_aug[t][:, 65 * u : 65 * (u + 1)],
                                start=(t == 0),
                                stop=(t == 15),
                            )

                for t in range(16):
                    et_half = []
                    for half in range(2):
                        ps_sc = psc.tile([128, 1024], F32, tag="sc")
                        for sq in range(2):
                            nc.tensor.matmul(
                                ps_sc[:, 512 * sq : 512 * (sq + 1)],
                                lhsT=kT[u][:, 128 * t : 128 * (t + 1)],
                                rhs=qT[u][:, 1024 * half + 512 * sq : 1024 * half + 512 * (sq + 1)],
                                start=True,
                                stop=True,
                            )
                        if half == 1 and t % 2 == 0:
                            # offload ~1/4 of the exps to DVE via the
                            # exponent-field trick: bf16(int16(A*s + B)) ~=
                            # exp(s/8); softmax normalization cancels the
                            # systematic error (validated: <2e-4 effect on
                            # final rel err)
                            e16 = et_pool.tile(
                                [128, 1024], mybir.dt.uint16, tag="et",
                                name=f"e16_{u}_{t}",
                            )
                            nc.vector.tensor_scalar(
                                out=e16,
                                in0=ps_sc,
                                scalar1=0.125 * 128.0 / 0.6931471805599453,
                                scalar2=16256.0 - 6.5,
                                op0=ALU.mult,
                                op1=ALU.add,
                            )
                            et_half.append((e16, True))
                        else:
                            etb = et_pool.tile([128, 1024], BF16, tag="et")
                            nc.scalar.activation(
                                out=etb, in_=ps_sc, func=AF.Exp,
                                bias=0.0, scale=0.125,
                            )
                            et_half.append((etb, False))
                    if et_prev is not None:
                        emit_av(t - 1, et_prev)
                    et_prev = et_half
                emit_av(15, et_prev)

                # drain: recips first, then normalize straight into the
                # token-major ship tile (frees PSUM fast; no transposes here
                # - the receiver transposes during the exchange window)
                o_tok = ot_pool.tile([128, 1024], BF16, tag="oT", name=f"oT{u}")
                recs = []
                for s_i in range(16):
                    g, jj = s_i // 4, s_i % 4
                    rec = small.tile(
                        [128, 1], F32, tag=f"rec{s_i}", name=f"rc{u}_{s_i}"
                    )
                    with nc.allow_low_precision(reason="softmax recip"):
                        nc.vector.reciprocal(
                            rec, o_ps[g][:, 65 * jj + 64 : 65 * jj + 65]
                        )
                    recs.append(rec)
                for s_i in range(16):
                    g, jj = s_i // 4, s_i % 4
                    if s_i % 2 == 0:
                        nc.vector.tensor_scalar(
                            out=o_tok[:, 64 * s_i : 64 * (s_i + 1)],
                            in0=o_ps[g][:, 65 * jj : 65 * jj + 64],
                            scalar1=recs[s_i],
                            scalar2=None,
                            op0=ALU.mult,
                        )
                    else:
                        # ACT is idle during the drain; Identity with a
                        # per-partition scale does the same normalize
                        nc.scalar.activation(
                            out=o_tok[:, 64 * s_i : 64 * (s_i + 1)],
                            in_=o_ps[g][:, 65 * jj : 65 * jj + 64],
                            func=AF.Identity,
                            bias=0.0,
                            scale=recs[s_i],
                        )
                nc.sync.dma_start(
                    out=send_h[u][:].rearrange("(j p) c -> p j c", p=128),
                    in_=o_tok[:].rearrange("p (j c) -> p j c", j=8),
                )
                nc.gpsimd.collective_compute(
                    "AllToAll",
                    ALU.bypass,
                    replica_groups=GROUPS,
                    ins=[send_h[u][:].opt()],
                    outs=[recv_h[u][:].opt()],
                )

            # ---- token phase ----
            # unit A's blocks landed during unit B's attention; loads and
            # transposes are emitted after unit B so their pool-slot requests
            # queue behind unit B's and naturally fill the exchange window
            rtA = oc_pool.tile([128, 1024], BF16, tag="rtA", name="rtA")
            nc.sync.dma_start(
                out=rtA[:].rearrange("p (j c) -> p j c", c=128),
                in_=recv_h[0][:].rearrange("(j p) c -> p j c", p=128)
            )
            ocA = [
                oc_pool.tile([64, HSH], BF16, tag=f"ocA{s}", name=f"ocA{s}")
                for s in range(8)
            ]
            for s in range(8):
                for hf in range(2):
                    pt = psc.tile([64, 128], BF16, tag="sc", name=f"ptA{s}_{hf}")
                    nc.tensor.transpose(
                        pt, rtA[:, 128 * s + 64 * hf : 128 * s + 64 * (hf + 1)], ident
                    )
                    nc.vector.tensor_copy(
                        ocA[s][:, 128 * hf : 128 * (hf + 1)], pt
                    )
            # Wo in two K=64 passes: pass 1 (even heads, from the first
            # exchange) fills the second AllToAll's latency window; pass 2
            # (odd heads) runs once the second exchange lands.
            ps_wo = [
                pacc.tile([128, 512], F32, tag="acc", name=f"pswo{i}")
                for i in range(4)
            ]
            last_p1 = None
            for i in range(4):
                bh, il = i // 2, i % 2
                for r in range(4):
                    last_p1 = nc.tensor.matmul(
                        ps_wo[i],
                        lhsT=ocA[4 * bh + r][:, 128 * il : 128 * (il + 1)],
                        rhs=woe_sb[:, 512 * r : 512 * (r + 1)],
                        start=(r == 0),
                        stop=False,
                    )
            # bridge the second AllToAll's latency so the FFN starts warm
            pe_warm(44, "a2a", pin_after=last_p1)
            rtB = oc_pool.tile([128, 1024], BF16, tag="rtB", name="rtB")
            nc.scalar.dma_start(
                out=rtB[:].rearrange("p (j c) -> p j c", c=128),
                in_=recv_h[1][:].rearrange("(j p) c -> p j c", p=128)
            )
            ocB = [
                oc_pool.tile([64, HSH], BF16, tag=f"ocB{s}", name=f"ocB{s}")
                for s in range(8)
            ]
            for s in range(8):
                for hf in range(2):
                    pt = psc.tile([64, 128], BF16, tag="sc", name=f"ptB{s}_{hf}")
                    nc.tensor.transpose(
                        pt, rtB[:, 128 * s + 64 * hf : 128 * s + 64 * (hf + 1)], ident
                    )
                    nc.vector.tensor_copy(
                        ocB[s][:, 128 * hf : 128 * (hf + 1)], pt
                    )

            def ln_core(dst, src):
                # (x - mu) * rsqrt(var + eps); gamma/beta folded elsewhere
                st = small.tile([128, 6], F32, tag="st")
                nc.vector.bn_stats(st, src)
                mv = small.tile([128, 2], F32, tag="mv")
                nc.vector.bn_aggr(mv, st)
                rstd = small.tile([128, 1], F32, tag="rstd")
                nc.scalar.activation(
                    out=rstd, in_=mv[:, 1:2], func=AF.Sqrt, bias=eps_t, scale=1.0
                )
                nc.vector.reciprocal(rstd, rstd)
                nmr = small.tile([128, 1], F32, tag="nmr")
                nc.vector.tensor_scalar(
                    out=nmr,
                    in0=mv[:, 0:1],
                    scalar1=rstd,
                    scalar2=-1.0,
                    op0=ALU.mult,
                    op1=ALU.mult,
                )
                nc.scalar.activation(
                    out=dst, in_=src, func=AF.Identity, bias=nmr, scale=rstd
                )
                return rstd, nmr

            h_bf = [None] * 4
            hT = [
                h_pool.tile([128, 512], BF16, tag=f"hT{dd}", name=f"hT{dd}")
                for dd in range(4)
            ]
            last_p2 = None
            for i in range(4):
                # token chunk i: chunks 0,1 = batch 0's 256 tokens; 2,3 = batch 1
                bh, il = i // 2, i % 2
                for r in range(4):
                    last_p2 = nc.tensor.matmul(
                        ps_wo[i],
                        lhsT=ocB[4 * bh + r][:, 128 * il : 128 * (il + 1)],
                        rhs=woo_sb[:, 512 * r : 512 * (r + 1)],
                        start=False,
                        stop=(r == 3),
                    )
                if i == 3:
                    # bridge the LN1 chains so the FFN enters at full clock
                    pe_warm(22, "ln1", pin_after=last_p2)
                t1 = tmp_pool.tile([128, D], F32, tag="t1")
                nc.vector.tensor_tensor(out=t1, in0=ps_wo[i], in1=xsbo[i], op=ALU.add)
                hb = h_pool.tile([128, D], BF16, tag=f"h{i}", name=f"h{i}")
                ln_core(hb, t1)
                h_bf[i] = hb
                for dd in range(4):
                    pt = pacc.tile([128, 128], BF16, tag="acc", name=f"ph{i}_{dd}")
                    nc.tensor.transpose(pt, hb[:, 128 * dd : 128 * (dd + 1)], ident)
                    nc.vector.tensor_copy(hT[dd][:, 128 * i : 128 * (i + 1)], pt)

            ff_ps = [
                pacc.tile([128, 512], F32, tag="acc", name=f"ff{i}") for i in range(4)
            ]
            for f in range(16):
                ps1 = psc.tile([128, 512], F32, tag="sc", name=f"ps1_{f}")
                for dd in range(4):
                    nc.tensor.matmul(
                        ps1,
                        lhsT=w1_sb[:, 2048 * dd + 128 * f : 2048 * dd + 128 * (f + 1)],
                        rhs=hT[dd],
                        start=(dd == 0),
                        stop=(dd == 3),
                    )
                f1 = f1_pool.tile([128, 512], BF16, tag="f1", name=f"f1_{f}")
                if f % 2 == 0:
                    nc.scalar.activation(
                        out=f1, in_=ps1, func=AF.Relu,
                        bias=cst_sb[:, 2 + f : 3 + f], scale=1.0,
                    )
                else:
                    # alternate relu between ACT and DVE to overlap two
                    # f-chunks' activations and keep the FFN matmuls dense
                    nc.vector.tensor_scalar(
                        out=f1,
                        in0=ps1,
                        scalar1=cst_sb[:, 2 + f : 3 + f],
                        scalar2=0.0,
                        op0=ALU.add,
                        op1=ALU.max,
                    )
                for i in range(4):
                    nc.tensor.matmul(
                        ff_ps[i],
                        lhsT=f1[:, 128 * i : 128 * (i + 1)],
                        rhs=w2_sb[:, 512 * f : 512 * (f + 1)],
                        start=(f == 0),
                        stop=False,
                    )
            for i in range(4):
                nc.tensor.matmul(
                    ff_ps[i], lhsT=ones128, rhs=b2r, start=False, stop=True
                )

            for i in range(4):
                # residual 2: ff (+ b2 + ln1_b already accumulated) + h_core*g1
                hg = tmp_pool.tile([128, D], F32, tag="hg")
                nc.gpsimd.tensor_tensor(out=hg, in0=h_bf[i], in1=g1_t, op=ALU.mult)
                t2 = tmp_pool.tile([128, D], F32, tag="e2")
                nc.vector.tensor_tensor(out=t2, in0=ff_ps[i], in1=hg, op=ALU.add)
                tn = tmp_pool.tile([128, D], F32, tag="tn")
                ln_core(tn, t2)
                tg = tmp_pool.tile([128, D], F32, tag="tg")
                nc.vector.tensor_tensor(out=tg, in0=tn, in1=g2_t, op=ALU.mult)
                o_sb = tmp_pool.tile([128, D], F32, tag="osb")
                nc.gpsimd.tensor_tensor(out=o_sb, in0=tg, in1=be2_t, op=ALU.add)
                nc.sync.dma_start(out=out_h[128 * i : 128 * (i + 1), :], in_=o_sb)

    _split_excess_waits(nc)
    return nc


_NC_CACHE = {}


def _get_nc():
    if "nc" not in _NC_CACHE:
        _NC_CACHE["nc"] = build_nc()
    return _NC_CACHE["nc"]


def build_in_maps(inputs):
    bf = mybir.dt.np(BF16)
    x = np.asarray(inputs["x"], np.float32)
    Wq = np.asarray(inputs["Wq"], np.float32)
    bq = np.asarray(inputs["bq"], np.float32)
    Wk = np.asarray(inputs["Wk"], np.float32)
    bk = np.asarray(inputs["bk"], np.float32)
    Wv = np.asarray(inputs["Wv"], np.float32)
    bv = np.asarray(inputs["bv"], np.float32)
    Wo = np.asarray(inputs["Wo"], np.float32)
    bo = np.asarray(inputs["bo"], np.float32)
    ln1_g = np.asarray(inputs["ln1_g"], np.float32)
    ln1_b = np.asarray(inputs["ln1_b"], np.float32)
    W1 = np.asarray(inputs["W1"], np.float32)
    b1 = np.asarray(inputs["b1"], np.float32)
    W2 = np.asarray(inputs["W2"], np.float32)
    b2 = np.asarray(inputs["b2"], np.float32)
    ln2_g = np.asarray(inputs["ln2_g"], np.float32)
    ln2_b = np.asarray(inputs["ln2_b"], np.float32)

    def swz(w, p=128):
        # [K, N] -> [p, (K//p)*N] so SBUF partition lines are contiguous
        k, n = w.shape
        return np.ascontiguousarray(
            w.reshape(k // p, p, n).transpose(1, 0, 2).reshape(p, (k // p) * n)
        )

    # fold LN1 gamma into W1 rows and LN1 beta into the FFN bias / residual:
    #   relu((h*g1 + be1) @ W1 + b1) = relu(h @ (g1*W1) + (be1 @ W1 + b1))
    #   pre-LN2 sum gets + be1 via bb2 = b2 + be1 (ones-matmul path)
    w1s = swz(W1 * ln1_g[:, None]).astype(bf)
    b1f = b1 + ln1_b @ W1
    bb2 = (b2 + ln1_b).astype(np.float32)
    w2s = swz(W2).astype(bf)
    wo4 = Wo.reshape(4, 2, 64, D)
    woe = np.ascontiguousarray(
        wo4[:, 0].transpose(1, 0, 2).reshape(64, 4 * D)
    ).astype(bf)
    woo = np.ascontiguousarray(
        wo4[:, 1].transpose(1, 0, 2).reshape(64, 4 * D)
    ).astype(bf)
    xts = [np.ascontiguousarray(x[b].T).astype(bf) for b in range(B)]

    in_maps = []
    for c in range(N_CORES):
        b, r = c // 4, c % 4
        h0, h1 = 2 * r, 2 * r + 1
        wqk = swz(
            np.concatenate([Wq[h0], Wq[h1], Wk[h0], Wk[h1]], axis=1)
        ).astype(bf)
        wv2 = swz(np.concatenate([Wv[h0], Wv[h1]], axis=1)).astype(bf)
        cst = np.zeros((128, 18), np.float32)
        cst[0:64, 0] = bq[h0]
        cst[64:128, 0] = bq[h1]
        cst[0:64, 1] = bk[h0]
        cst[64:128, 1] = bk[h1]
        cst[:, 2:18] = b1f.reshape(16, 128).T
        bv2 = np.ascontiguousarray(
            np.broadcast_to(
                np.concatenate([bv[h0], bv[h1]])[None, :], (128, 128)
            )
        ).astype(np.float32)
        # token shard of core c: 256 tokens of batch 0 + 256 of batch 1
        xsb = (
            np.concatenate(
                [x[0, HSH * c : HSH * (c + 1)], x[1, HSH * c : HSH * (c + 1)]]
            )
            + bo[None, :]
        ).astype(bf)
        in_maps.append(
            {
                "xt": xts[b],
                "wqk": wqk,
                "wv": wv2,
                "woe": woe,
                "woo": woo,
                "w1": w1s,
                "w2": w2s,
                "xsb": xsb,
                "cst": cst,
                "bv": bv2,
                "g1": ln1_g,
                "g2": ln2_g,
                "be2": ln2_b,
                "b2": bb2,
            }
        )
    return in_maps


def kernel(**inputs):
    in_maps = build_in_maps(inputs)
    nc = _get_nc()
    res = run_bass_kernel_spmd(nc, in_maps, core_ids=list(range(N_CORES)))
    out = np.empty((B, S, D), np.float32)
    for c in range(N_CORES):
        sh = res.results[c]["out_shard"]
        out[0, HSH * c : HSH * (c + 1)] = sh[:HSH]
        out[1, HSH * c : HSH * (c + 1)] = sh[HSH:]
    return out
